# revision 2
# baseline (speedup 1.0000x reference)
"""Causal multi-head attention on 8 trn2 NeuronCores — v2 (all-bf16, no spills).

Problem: B=4, S=2048, D=2048, H=16 heads, head_dim=128, causal softmax,
torch-style Linear projections (W stored [in, out]).

Sharding: core c handles batch b = c//2 and head-group g = c%2
(8 heads = 1024 output columns of Wq/Wk/Wv, 1024 rows of Wo).
Each core produces a partial output [S, D]; host sums the two
head-group partials per batch and adds bo.

v2 design (vs v1): all operands bf16 (error budget allows: sim 3.6e-3
vs 2e-2 gate), which halves SBUF footprint so Q^T/K^T/V stay fully
SBUF-resident — no DRAM spill/reload.  The output projection is fused
into the attention loop per 512-wide q-chunk (ctx tiles consumed from
SBUF), with its matmuls interleaved between heads of the next q-chunk
as PE filler during ACT-exp waits.  Exp is batched over tile pairs
[128, 2, 512] to halve ACT instruction count.
"""

import numpy as np
import ml_dtypes

import concourse.bass as bass
import concourse.mybir as mybir
import concourse.tile as tile
from concourse import bacc
from concourse.bass_utils import run_bass_kernel_spmd

B = 4
S = 2048
D = 2048
H = 16
DH = 128
HPC = 8          # heads per core
DHG = HPC * DH   # 1024: head-group width per core
KT = D // 128    # 16 k-tiles over the model dim
ST = S // 128    # 16 s-tiles
QC = S // 512    # 4 q-chunks
SCALE = 1.0 / np.sqrt(DH)
NEG = -1.0e30

F32 = mybir.dt.float32
BF16 = mybir.dt.bfloat16
AF = mybir.ActivationFunctionType


def _build_nc():
    nc = bacc.Bacc(None, target_bir_lowering=False)

    xT = nc.declare_dram_parameter("xT", [D, S], BF16, isOutput=False)
    # wq/wk host-pregathered to [HPC*128, KT*128]: row t*128+p, col n*128+m
    # = Wq[n*128+p, t*128+m] so each head-tile's weights DMA contiguously
    wq = nc.declare_dram_parameter("wq", [DHG, D], BF16, isOutput=False)
    wk = nc.declare_dram_parameter("wk", [DHG, D], BF16, isOutput=False)
    wv = nc.declare_dram_parameter("wv", [D, DHG], BF16, isOutput=False)
    wo = nc.declare_dram_parameter("wo", [128, HPC, D], BF16, isOutput=False)
    bqT = nc.declare_dram_parameter("bqT", [128, HPC], F32, isOutput=False)
    bkT = nc.declare_dram_parameter("bkT", [128, HPC], F32, isOutput=False)
    bvb = nc.declare_dram_parameter("bvb", [128, DHG], F32, isOutput=False)
    cmask = nc.declare_dram_parameter("cmask", [128, 896], F32, isOutput=False)
    out = nc.declare_dram_parameter("out", [S, D], F32, isOutput=True)

    with tile.TileContext(nc) as tc:
        _emit(nc, tc, xT, wq, wk, wv, wo, bqT, bkT, bvb, cmask, out)
    nc.compile()
    return nc


def _emit(nc, tc, xT, wq, wk, wv, wo, bqT, bkT, bvb, cmask, out):
    with (
        tc.tile_pool(name="const", bufs=1) as const,
        tc.tile_pool(name="persist", bufs=1) as persist,
    ):
        # small consts needed early
        bq_sb = const.tile([128, HPC], F32)
        nc.sync.dma_start(out=bq_sb, in_=bqT[:, :])
        bk_sb = const.tile([128, HPC], F32)
        nc.sync.dma_start(out=bk_sb, in_=bkT[:, :])
        ones_bf = const.tile([128, 128], BF16)
        nc.vector.memset(ones_bf, 1.0)

        qt_all = persist.tile([128, HPC, S], BF16)
        kt_all = persist.tile([128, HPC, S], BF16)
        v_all = persist.tile([128, ST, DHG], BF16)
        wo_sb = persist.tile([128, HPC, D], BF16)
        cm_sb = const.tile([128, 896], F32)
        bv_sb = const.tile([128, DHG], F32)

        # ---------------- Phase A: projections, SBUF-resident -------------
        wv_r = wv.rearrange("(n p) m -> p n m", p=128)

        with (
            tc.tile_pool(name="xts", bufs=3) as xtp,
            tc.tile_pool(name="wqk", bufs=2) as wqk,
            tc.tile_pool(name="wvp", bufs=2) as wvp,
            tc.tile_pool(name="apsum", bufs=8, space="PSUM") as aps,
        ):
            for sh in range(2):
                s0 = sh * (S // 2)
                seq = [(w, b, dst, t)
                       for w, b, dst in ((wq, bq_sb, qt_all), (wk, bk_sb, kt_all))
                       for t in range(HPC)]
                w_tiles = {}

                def w_prefetch(i):
                    if i < len(seq):
                        w, _, _, t = seq[i]
                        w_sb = wqk.tile([128, KT, 128], BF16, tag="wqk",
                                        name=f"w_sb{i % 2}")
                        nc.sync.dma_start(
                            out=w_sb,
                            in_=w[t * 128 : (t + 1) * 128, :]
                            .rearrange("p (n m) -> p n m", m=128),
                        )
                        w_tiles[i] = w_sb

                w_prefetch(0)

                xt_lo = xtp.tile([128, 8, S // 2], BF16, tag="xts")
                xt_hi = xtp.tile([128, 8, S // 2], BF16, tag="xts")

                def xt_blk(kd):
                    t = xt_lo if kd < 8 else xt_hi
                    return t[:, kd % 8, :]

                for kd in range(KT):
                    nc.sync.dma_start(
                        out=xt_blk(kd),
                        in_=xT[kd * 128 : (kd + 1) * 128, s0 : s0 + S // 2],
                    )
                if sh == 0:
                    # bv needed by the V staging later this half
                    nc.sync.dma_start(out=bv_sb, in_=bvb[:, :])

                # Q^T and K^T: psum[dh 128, s 512] = sum_kd Wblk^T @ xTblk,
                # bias-add on ACT straight into the resident buffers
                for i, (w, b_sb, dst, t) in enumerate(seq):
                    w_sb = w_tiles.pop(i)
                    w_prefetch(i + 1)
                    for sc in range(2):
                        psum = aps.tile([128, 512], F32, tag="apsum", name="qk_ps")
                        for kd in range(KT):
                            nc.tensor.matmul(
                                psum,
                                w_sb[:, kd, :],
                                xt_blk(kd)[:, sc * 512 : (sc + 1) * 512],
                                start=(kd == 0),
                                stop=(kd == KT - 1),
                            )
                        nc.scalar.activation(
                            out=dst[:, t, s0 + sc * 512 : s0 + (sc + 1) * 512],
                            in_=psum,
                            func=AF.Identity,
                            bias=b_sb[:, t : t + 1],
                        )

                # V: psum[s 128, dh 512] = sum_kd xTblk^T @ Wvblk
                for t2 in range(2):
                    wv_lo = wvp.tile([128, 8, 512], BF16, tag="wvp", name="wv_lo")
                    nc.sync.dma_start(
                        out=wv_lo, in_=wv_r[:, 0:8, t2 * 512 : (t2 + 1) * 512]
                    )
                    wv_hi = wvp.tile([128, 8, 512], BF16, tag="wvp", name="wv_hi")
                    nc.sync.dma_start(
                        out=wv_hi, in_=wv_r[:, 8:16, t2 * 512 : (t2 + 1) * 512]
                    )
                    psums = [
                        aps.tile([128, 512], F32, tag="apsum", name=f"vps{si}")
                        for si in range(8)
                    ]
                    for kd in range(KT):
                        wv_blk = wv_lo if kd < 8 else wv_hi
                        for si in range(8):
                            nc.tensor.matmul(
                                psums[si],
                                xt_blk(kd)[:, si * 128 : (si + 1) * 128],
                                wv_blk[:, kd % 8, :],
                                start=(kd == 0),
                                stop=(kd == KT - 1),
                            )
                    for si in range(8):
                        nc.vector.tensor_tensor(
                            out=v_all[:, sh * 8 + si, t2 * 512 : (t2 + 1) * 512],
                            in0=psums[si],
                            in1=bv_sb[:, t2 * 512 : (t2 + 1) * 512],
                            op=mybir.AluOpType.add,
                        )

            # mask needed only at phase B start; queue behind the bulk loads
            nc.sync.dma_start(out=cm_sb, in_=cmask[:, :])

        # ---------------- Phase B + fused C, per q-chunk ---------------------
        with (
            tc.tile_pool(name="msk", bufs=3) as mpool,
            tc.tile_pool(name="ptile", bufs=4) as ppool,
            tc.tile_pool(name="psum8", bufs=3) as spool,
            tc.tile_pool(name="ctx", bufs=2) as ctxp,
            tc.tile_pool(name="rcp", bufs=2) as rcpool,
            tc.tile_pool(name="ostage", bufs=4) as ost,
            tc.tile_pool(name="pscore", bufs=3, space="PSUM") as pscore,
            tc.tile_pool(name="pcd", bufs=1, space="PSUM") as pcd,
        ):
            for hh in range(HPC):
                nc.sync.dma_start(out=wo_sb[:, hh, :], in_=wo[:, hh, :])

            cfill = []  # deferred phase-C psum jobs of the previous q-chunk
            cslot = []  # half-used pscore pair tile shared by job pairs

            def c_job(ctx_t, st, st_rel, ncol):
                def run():
                    # C psums ride the pscore pool, two jobs per pair tile
                    if cslot:
                        psC = cslot.pop()
                    else:
                        pt = pscore.tile([128, 2, 512], F32, tag="pscore")
                        cslot.append(pt[:, 1, :])
                        psC = pt[:, 0, :]
                    for hh in range(HPC):
                        nc.tensor.matmul(
                            psC,
                            ctx_t[:, hh, st_rel * 128 : (st_rel + 1) * 128],
                            wo_sb[:, hh, ncol * 512 : (ncol + 1) * 512],
                            start=(hh == 0),
                            stop=(hh == HPC - 1),
                        )
                    o_sb = ost.tile([128, 512], F32, tag="ostage")
                    nc.scalar.activation(out=o_sb, in_=psC, func=AF.Copy)
                    # alternate store queues so the final drain isn't gated
                    # on a single DMA engine
                    eng = nc.gpsimd if ncol % 2 == 0 else nc.sync
                    eng.dma_start(
                        out=out[
                            st * 128 : (st + 1) * 128,
                            ncol * 512 : (ncol + 1) * 512,
                        ],
                        in_=o_sb,
                    )
                return run

            # PE-heavy chunks first: the small chunks (ACT-latency-bound)
            # then get output-projection filler work; qc0's own projection
            # forms the tail either way
            for qc in (2, 3, 1, 0):
                # for large chunks, pre-reduce P pairs on DVE so one
                # ones-matmul per pair suffices for the denominator
                pair_den = qc >= 2
                ctx_all = ctxp.tile([128, HPC, 512], BF16, tag="ctx")
                # tile pairs: diagonal pairs first (long PE->DVE->ACT chains
                # start early; first ctx matmul covers the full psum width)
                pairs = [
                    (4 * qc, 0, 4 * qc + 1, 128, True),
                    (4 * qc + 2, 256, 4 * qc + 3, 384, True),
                ] + [(2 * i, 0, 2 * i + 1, 0, False) for i in range(2 * qc)]

                for h in range(HPC):
                    qt_c = qt_all[:, h, qc * 512 : (qc + 1) * 512]
                    psum_cd = pcd.tile([128, 2, 512], F32, tag="pcd")

                    def scores_exp(pr):
                        ka, offa, kb, offb, diag = pr
                        ps = pscore.tile([128, 2, 512], F32, tag="pscore")
                        nc.tensor.matmul(
                            ps[:, 0, offa:],
                            kt_all[:, h, ka * 128 : (ka + 1) * 128],
                            qt_c[:, offa:],
                            start=True, stop=True,
                        )
                        nc.tensor.matmul(
                            ps[:, 1, offb:],
                            kt_all[:, h, kb * 128 : (kb + 1) * 128],
                            qt_c[:, offb:],
                            start=True, stop=True,
                        )
                        p_t = ppool.tile([128, 2, 512], BF16, tag="p_t")
                        if diag:
                            ja, jb = ka - 4 * qc, kb - 4 * qc
                            msk = mpool.tile([128, 2, 512], F32, tag="msk")
                            nc.vector.tensor_tensor(
                                out=msk[:, 0, offa:],
                                in0=ps[:, 0, offa:],
                                in1=cm_sb[:, 384 + offa - 128 * ja : 896 - 128 * ja],
                                op=mybir.AluOpType.add,
                            )
                            nc.vector.tensor_tensor(
                                out=msk[:, 1, offb:],
                                in0=ps[:, 1, offb:],
                                in1=cm_sb[:, 384 + offb - 128 * jb : 896 - 128 * jb],
                                op=mybir.AluOpType.add,
                            )
                            # cols [offa:offb) of the odd slot are fully masked
                            nc.vector.memset(msk[:, 1, offa:offb], NEG)
                            src = msk
                        else:
                            src = ps
                        nc.scalar.activation(
                            out=p_t[:, :, offa:],
                            in_=src[:, :, offa:],
                            func=AF.Exp,
                            scale=float(SCALE),
                        )
                        if pair_den:
                            # masked cols of the odd slot hold exact zeros,
                            # so the pair sum is valid from offa on
                            s_t = spool.tile([128, 512], BF16, tag="psum8")
                            nc.vector.tensor_tensor(
                                out=s_t[:, offa:],
                                in0=p_t[:, 0, offa:],
                                in1=p_t[:, 1, offa:],
                                op=mybir.AluOpType.add,
                            )
                        else:
                            s_t = None
                        return p_t, s_t

                    def ctx_mm(pr, p_t, s_t, first, last):
                        ka, offa, kb, offb, _ = pr
                        nc.tensor.matmul(
                            psum_cd[:, 0, offa:],
                            v_all[:, ka, h * 128 : (h + 1) * 128],
                            p_t[:, 0, offa:],
                            start=first, stop=False,
                        )
                        if not pair_den:
                            nc.tensor.matmul(
                                psum_cd[:, 1, offa:],
                                ones_bf,
                                p_t[:, 0, offa:],
                                start=first, stop=False,
                            )
                        nc.tensor.matmul(
                            psum_cd[:, 0, offb:],
                            v_all[:, kb, h * 128 : (h + 1) * 128],
                            p_t[:, 1, offb:],
                            start=False, stop=last,
                        )
                        if pair_den:
                            nc.tensor.matmul(
                                psum_cd[:, 1, offa:],
                                ones_bf,
                                s_t[:, offa:],
                                start=first, stop=last,
                            )
                        else:
                            nc.tensor.matmul(
                                psum_cd[:, 1, offb:],
                                ones_bf,
                                p_t[:, 1, offb:],
                                start=False, stop=last,
                            )

                    prev = None
                    for i, pr in enumerate(pairs):
                        p_t, s_t = scores_exp(pr)
                        if prev is not None:
                            ctx_mm(prev[0], prev[1], prev[2],
                                   first=(i == 1), last=False)
                        prev = (pr, p_t, s_t)
                    ctx_mm(prev[0], prev[1], prev[2],
                           first=(len(pairs) == 1), last=True)

                    recip = rcpool.tile([128, 512], F32, tag="rcp")
                    nc.vector.reciprocal_approx_fast(out=recip, in_=psum_cd[:, 1, :])
                    nc.vector.tensor_tensor(
                        out=ctx_all[:, h, :],
                        in0=psum_cd[:, 0, :],
                        in1=recip,
                        op=mybir.AluOpType.mult,
                    )
                    # PE filler: two output-projection psums of the previous
                    # q-chunk ride along each head iteration
                    for _ in range(2):
                        if cfill:
                            cfill.pop(0)()

                for st_rel in range(4):
                    st = 4 * qc + st_rel
                    for ncol in range(4):
                        cfill.append(c_job(ctx_all, st, st_rel, ncol))
            while cfill:
                cfill.pop(0)()


_NC = None


def _get_nc():
    global _NC
    if _NC is None:
        _NC = _build_nc()
    return _NC


def _host_prep(input_sequences, Wq, bq, Wk, bk, Wv, bv, Wo, bo):
    """Build per-core input maps (bf16 conversion + pre-gather on host)."""
    bf = ml_dtypes.bfloat16
    x = np.asarray(input_sequences, dtype=np.float32)
    cm = np.full((128, 896), NEG, dtype=np.float32)
    kk = np.arange(128)[:, None]
    uu = np.arange(896)[None, :]
    cm[kk <= uu - 384] = 0.0

    in_maps = []
    for c in range(8):
        b, g = divmod(c, 2)
        sl = slice(g * DHG, (g + 1) * DHG)
        wq_c = np.ascontiguousarray(
            np.asarray(Wq[:, sl], dtype=np.float32)
            .reshape(KT, 128, HPC, 128).transpose(2, 1, 0, 3).reshape(DHG, D)
        ).astype(bf)
        wk_c = np.ascontiguousarray(
            np.asarray(Wk[:, sl], dtype=np.float32)
            .reshape(KT, 128, HPC, 128).transpose(2, 1, 0, 3).reshape(DHG, D)
        ).astype(bf)
        wv_c = np.ascontiguousarray(Wv[:, sl]).astype(np.float32).astype(bf)
        wo_c = np.ascontiguousarray(
            np.asarray(Wo[sl, :], dtype=np.float32)
            .reshape(HPC, 128, D).transpose(1, 0, 2)
        ).astype(bf)
        in_maps.append({
            "xT": np.ascontiguousarray(x[b].T).astype(bf),
            "wq": wq_c,
            "wk": wk_c,
            "wv": wv_c,
            "wo": wo_c,
            "bqT": np.ascontiguousarray(
                np.asarray(bq[sl], dtype=np.float32).reshape(HPC, 128).T
            ),
            "bkT": np.ascontiguousarray(
                np.asarray(bk[sl], dtype=np.float32).reshape(HPC, 128).T
            ),
            "bvb": np.ascontiguousarray(
                np.broadcast_to(np.asarray(bv[sl], dtype=np.float32), (128, DHG))
            ),
            "cmask": cm,
        })
    return in_maps


def kernel(input_sequences, Wq, bq, Wk, bk, Wv, bv, Wo, bo, _trace=False):
    nc = _get_nc()
    in_maps = _host_prep(input_sequences, Wq, bq, Wk, bk, Wv, bv, Wo, bo)
    res = run_bass_kernel_spmd(nc, in_maps, list(range(8)), trace=_trace)
    bo32 = np.asarray(bo, dtype=np.float32)
    out = np.empty((B, S, D), dtype=np.float32)
    for b in range(B):
        out[b] = res.results[2 * b]["out"] + res.results[2 * b + 1]["out"] + bo32
    if _trace:
        kernel.last_exec_time_ns = res.exec_time_ns
    return out


# revision 3
# speedup vs baseline: 1.0108x; 1.0108x over previous
"""Causal multi-head attention on 8 trn2 NeuronCores — v2 (all-bf16, no spills).

Problem: B=4, S=2048, D=2048, H=16 heads, head_dim=128, causal softmax,
torch-style Linear projections (W stored [in, out]).

Sharding: core c handles batch b = c//2 and head-group g = c%2
(8 heads = 1024 output columns of Wq/Wk/Wv, 1024 rows of Wo).
Each core produces a partial output [S, D]; host sums the two
head-group partials per batch and adds bo.

v2 design (vs v1): all operands bf16 (error budget allows: sim 3.6e-3
vs 2e-2 gate), which halves SBUF footprint so Q^T/K^T/V stay fully
SBUF-resident — no DRAM spill/reload.  The output projection is fused
into the attention loop per 512-wide q-chunk (ctx tiles consumed from
SBUF), with its matmuls interleaved between heads of the next q-chunk
as PE filler during ACT-exp waits.  Exp is batched over tile pairs
[128, 2, 512] to halve ACT instruction count.
"""

import numpy as np
import ml_dtypes

import concourse.bass as bass
import concourse.mybir as mybir
import concourse.tile as tile
from concourse import bacc
from concourse.bass_utils import run_bass_kernel_spmd

B = 4
S = 2048
D = 2048
H = 16
DH = 128
HPC = 8          # heads per core
DHG = HPC * DH   # 1024: head-group width per core
KT = D // 128    # 16 k-tiles over the model dim
ST = S // 128    # 16 s-tiles
QC = S // 512    # 4 q-chunks
SCALE = 1.0 / np.sqrt(DH)
NEG = -1.0e30

F32 = mybir.dt.float32
BF16 = mybir.dt.bfloat16
AF = mybir.ActivationFunctionType


def _build_nc():
    nc = bacc.Bacc(None, target_bir_lowering=False)

    xT = nc.declare_dram_parameter("xT", [D, S], BF16, isOutput=False)
    # wq/wk host-pregathered to [HPC*128, KT*128]: row t*128+p, col n*128+m
    # = Wq[n*128+p, t*128+m] so each head-tile's weights DMA contiguously
    wq = nc.declare_dram_parameter("wq", [DHG, D], BF16, isOutput=False)
    wk = nc.declare_dram_parameter("wk", [DHG, D], BF16, isOutput=False)
    wv = nc.declare_dram_parameter("wv", [D, DHG], BF16, isOutput=False)
    wo = nc.declare_dram_parameter("wo", [128, HPC, D], BF16, isOutput=False)
    bqT = nc.declare_dram_parameter("bqT", [128, HPC], F32, isOutput=False)
    bkT = nc.declare_dram_parameter("bkT", [128, HPC], F32, isOutput=False)
    bvb = nc.declare_dram_parameter("bvb", [128, DHG], F32, isOutput=False)
    cmask = nc.declare_dram_parameter("cmask", [128, 896], F32, isOutput=False)
    out = nc.declare_dram_parameter("out", [S, D], F32, isOutput=True)

    with tile.TileContext(nc) as tc:
        _emit(nc, tc, xT, wq, wk, wv, wo, bqT, bkT, bvb, cmask, out)
    nc.compile()
    return nc


def _emit(nc, tc, xT, wq, wk, wv, wo, bqT, bkT, bvb, cmask, out):
    with (
        tc.tile_pool(name="const", bufs=1) as const,
        tc.tile_pool(name="persist", bufs=1) as persist,
    ):
        # small consts needed early
        bq_sb = const.tile([128, HPC], F32)
        nc.sync.dma_start(out=bq_sb, in_=bqT[:, :])
        bk_sb = const.tile([128, HPC], F32)
        nc.sync.dma_start(out=bk_sb, in_=bkT[:, :])
        ones_bf = const.tile([128, 128], BF16)
        nc.vector.memset(ones_bf, 1.0)

        qt_all = persist.tile([128, HPC, S], BF16)
        kt_all = persist.tile([128, HPC, S], BF16)
        v_all = persist.tile([128, ST, DHG], BF16)
        wo_sb = persist.tile([128, HPC, D], BF16)
        cm_sb = const.tile([128, 896], F32)
        bv_sb = const.tile([128, DHG], F32)

        # ---------------- Phase A: projections, SBUF-resident -------------
        wv_r = wv.rearrange("(n p) m -> p n m", p=128)

        with (
            tc.tile_pool(name="xts", bufs=3) as xtp,
            tc.tile_pool(name="wqk", bufs=2) as wqk,
            tc.tile_pool(name="wvp", bufs=2) as wvp,
            tc.tile_pool(name="apsum", bufs=8, space="PSUM") as aps,
        ):
            for sh in range(2):
                s0 = sh * (S // 2)
                seq = [(w, b, dst, t)
                       for w, b, dst in ((wq, bq_sb, qt_all), (wk, bk_sb, kt_all))
                       for t in range(HPC)]
                w_tiles = {}

                def w_prefetch(i):
                    if i < len(seq):
                        w, _, _, t = seq[i]
                        w_sb = wqk.tile([128, KT, 128], BF16, tag="wqk",
                                        name=f"w_sb{i % 2}")
                        nc.sync.dma_start(
                            out=w_sb,
                            in_=w[t * 128 : (t + 1) * 128, :]
                            .rearrange("p (n m) -> p n m", m=128),
                        )
                        w_tiles[i] = w_sb

                w_prefetch(0)

                xt_lo = xtp.tile([128, 8, S // 2], BF16, tag="xts")
                xt_hi = xtp.tile([128, 8, S // 2], BF16, tag="xts")

                def xt_blk(kd):
                    t = xt_lo if kd < 8 else xt_hi
                    return t[:, kd % 8, :]

                for kd in range(KT):
                    nc.sync.dma_start(
                        out=xt_blk(kd),
                        in_=xT[kd * 128 : (kd + 1) * 128, s0 : s0 + S // 2],
                    )
                if sh == 0:
                    # bv needed by the V staging later this half
                    nc.sync.dma_start(out=bv_sb, in_=bvb[:, :])

                # Q^T and K^T: psum[dh 128, s 512] = sum_kd Wblk^T @ xTblk,
                # bias-add on ACT straight into the resident buffers
                for i, (w, b_sb, dst, t) in enumerate(seq):
                    w_sb = w_tiles.pop(i)
                    w_prefetch(i + 1)
                    for sc in range(2):
                        psum = aps.tile([128, 512], F32, tag="apsum", name="qk_ps")
                        for kd in range(KT):
                            nc.tensor.matmul(
                                psum,
                                w_sb[:, kd, :],
                                xt_blk(kd)[:, sc * 512 : (sc + 1) * 512],
                                start=(kd == 0),
                                stop=(kd == KT - 1),
                            )
                        nc.scalar.activation(
                            out=dst[:, t, s0 + sc * 512 : s0 + (sc + 1) * 512],
                            in_=psum,
                            func=AF.Identity,
                            bias=b_sb[:, t : t + 1],
                        )

                # V: psum[s 128, dh 512] = sum_kd xTblk^T @ Wvblk
                for t2 in range(2):
                    wv_lo = wvp.tile([128, 8, 512], BF16, tag="wvp", name="wv_lo")
                    nc.sync.dma_start(
                        out=wv_lo, in_=wv_r[:, 0:8, t2 * 512 : (t2 + 1) * 512]
                    )
                    wv_hi = wvp.tile([128, 8, 512], BF16, tag="wvp", name="wv_hi")
                    nc.sync.dma_start(
                        out=wv_hi, in_=wv_r[:, 8:16, t2 * 512 : (t2 + 1) * 512]
                    )
                    psums = [
                        aps.tile([128, 512], F32, tag="apsum", name=f"vps{si}")
                        for si in range(8)
                    ]
                    for kd in range(KT):
                        wv_blk = wv_lo if kd < 8 else wv_hi
                        for si in range(8):
                            nc.tensor.matmul(
                                psums[si],
                                xt_blk(kd)[:, si * 128 : (si + 1) * 128],
                                wv_blk[:, kd % 8, :],
                                start=(kd == 0),
                                stop=(kd == KT - 1),
                            )
                    for si in range(8):
                        nc.vector.tensor_tensor(
                            out=v_all[:, sh * 8 + si, t2 * 512 : (t2 + 1) * 512],
                            in0=psums[si],
                            in1=bv_sb[:, t2 * 512 : (t2 + 1) * 512],
                            op=mybir.AluOpType.add,
                        )

            # mask needed only at phase B start; queue behind the bulk loads
            nc.sync.dma_start(out=cm_sb, in_=cmask[:, :])

        # ---------------- Phase B + fused C, per q-chunk ---------------------
        with (
            tc.tile_pool(name="msk", bufs=3) as mpool,
            tc.tile_pool(name="ptile", bufs=4) as ppool,
            tc.tile_pool(name="psum8", bufs=3) as spool,
            tc.tile_pool(name="ctx", bufs=2) as ctxp,
            tc.tile_pool(name="rcp", bufs=2) as rcpool,
            tc.tile_pool(name="ostage", bufs=4) as ost,
            tc.tile_pool(name="pscore", bufs=3, space="PSUM") as pscore,
            tc.tile_pool(name="pcd", bufs=1, space="PSUM") as pcd,
        ):
            for hh in range(HPC):
                nc.sync.dma_start(out=wo_sb[:, hh, :], in_=wo[:, hh, :])

            cfill = []  # deferred phase-C psum jobs of the previous q-chunk
            cslot = []  # half-used pscore pair tile shared by job pairs

            def c_job(ctx_t, st, st_rel, ncol):
                def run():
                    # C psums ride the pscore pool, two jobs per pair tile
                    if cslot:
                        psC = cslot.pop()
                    else:
                        pt = pscore.tile([128, 2, 512], F32, tag="pscore")
                        cslot.append(pt[:, 1, :])
                        psC = pt[:, 0, :]
                    for hh in range(HPC):
                        nc.tensor.matmul(
                            psC,
                            ctx_t[:, hh, st_rel * 128 : (st_rel + 1) * 128],
                            wo_sb[:, hh, ncol * 512 : (ncol + 1) * 512],
                            start=(hh == 0),
                            stop=(hh == HPC - 1),
                        )
                    o_sb = ost.tile([128, 512], F32, tag="ostage")
                    nc.scalar.activation(out=o_sb, in_=psC, func=AF.Copy)
                    # alternate store queues so the final drain isn't gated
                    # on a single DMA engine
                    eng = nc.gpsimd if ncol % 2 == 0 else nc.sync
                    eng.dma_start(
                        out=out[
                            st * 128 : (st + 1) * 128,
                            ncol * 512 : (ncol + 1) * 512,
                        ],
                        in_=o_sb,
                    )
                return run

            def emit_head(qc, h, ctx_all):
                # for large chunks, pre-reduce P pairs on DVE so one
                # ones-matmul per pair suffices for the denominator
                pair_den = qc >= 2
                pairs = [
                    (4 * qc, 0, 4 * qc + 1, 128, True),
                    (4 * qc + 2, 256, 4 * qc + 3, 384, True),
                ] + [(2 * i, 0, 2 * i + 1, 0, False) for i in range(2 * qc)]
                qt_c = qt_all[:, h, qc * 512 : (qc + 1) * 512]
                psum_cd = pcd.tile([128, 2, 512], F32, tag="pcd")

                def scores_exp(pr):
                    ka, offa, kb, offb, diag = pr
                    ps = pscore.tile([128, 2, 512], F32, tag="pscore")
                    nc.tensor.matmul(
                        ps[:, 0, offa:],
                        kt_all[:, h, ka * 128 : (ka + 1) * 128],
                        qt_c[:, offa:],
                        start=True, stop=True,
                    )
                    nc.tensor.matmul(
                        ps[:, 1, offb:],
                        kt_all[:, h, kb * 128 : (kb + 1) * 128],
                        qt_c[:, offb:],
                        start=True, stop=True,
                    )
                    p_t = ppool.tile([128, 2, 512], BF16, tag="p_t")
                    if diag:
                        ja, jb = ka - 4 * qc, kb - 4 * qc
                        msk = mpool.tile([128, 2, 512], F32, tag="msk")
                        nc.vector.tensor_tensor(
                            out=msk[:, 0, offa:],
                            in0=ps[:, 0, offa:],
                            in1=cm_sb[:, 384 + offa - 128 * ja : 896 - 128 * ja],
                            op=mybir.AluOpType.add,
                        )
                        nc.vector.tensor_tensor(
                            out=msk[:, 1, offb:],
                            in0=ps[:, 1, offb:],
                            in1=cm_sb[:, 384 + offb - 128 * jb : 896 - 128 * jb],
                            op=mybir.AluOpType.add,
                        )
                        nc.vector.memset(msk[:, 1, offa:offb], NEG)
                        src = msk
                    else:
                        src = ps
                    nc.scalar.activation(
                        out=p_t[:, :, offa:],
                        in_=src[:, :, offa:],
                        func=AF.Exp,
                        scale=float(SCALE),
                    )
                    if pair_den:
                        s_t = spool.tile([128, 512], BF16, tag="psum8")
                        nc.vector.tensor_tensor(
                            out=s_t[:, offa:],
                            in0=p_t[:, 0, offa:],
                            in1=p_t[:, 1, offa:],
                            op=mybir.AluOpType.add,
                        )
                    else:
                        s_t = None
                    return p_t, s_t

                def ctx_mm(pr, p_t, s_t, first, last):
                    ka, offa, kb, offb, _ = pr
                    nc.tensor.matmul(
                        psum_cd[:, 0, offa:],
                        v_all[:, ka, h * 128 : (h + 1) * 128],
                        p_t[:, 0, offa:],
                        start=first, stop=False,
                    )
                    if not pair_den:
                        nc.tensor.matmul(
                            psum_cd[:, 1, offa:],
                            ones_bf,
                            p_t[:, 0, offa:],
                            start=first, stop=False,
                        )
                    nc.tensor.matmul(
                        psum_cd[:, 0, offb:],
                        v_all[:, kb, h * 128 : (h + 1) * 128],
                        p_t[:, 1, offb:],
                        start=False, stop=last,
                    )
                    if pair_den:
                        nc.tensor.matmul(
                            psum_cd[:, 1, offa:],
                            ones_bf,
                            s_t[:, offa:],
                            start=first, stop=last,
                        )
                    else:
                        nc.tensor.matmul(
                            psum_cd[:, 1, offb:],
                            ones_bf,
                            p_t[:, 1, offb:],
                            start=False, stop=last,
                        )

                prev = None
                for i, pr in enumerate(pairs):
                    p_t, s_t = scores_exp(pr)
                    if prev is not None:
                        ctx_mm(prev[0], prev[1], prev[2],
                               first=(i == 1), last=False)
                    prev = (pr, p_t, s_t)
                ctx_mm(prev[0], prev[1], prev[2],
                       first=(len(pairs) == 1), last=True)

                recip = rcpool.tile([128, 512], F32, tag="rcp")
                nc.vector.reciprocal_approx_fast(out=recip, in_=psum_cd[:, 1, :])
                nc.vector.tensor_tensor(
                    out=ctx_all[:, h, :],
                    in0=psum_cd[:, 0, :],
                    in1=recip,
                    op=mybir.AluOpType.mult,
                )

            for qc in (2, 3, 1, 0):
                ctx_all = ctxp.tile([128, HPC, 512], BF16, tag="ctx", name="ctxl")
                for h in range(HPC):
                    emit_head(qc, h, ctx_all)
                    for _ in range(2):
                        if cfill:
                            cfill.pop(0)()
                for st_rel in range(4):
                    st = 4 * qc + st_rel
                    for ncol in range(4):
                        cfill.append(c_job(ctx_all, st, st_rel, ncol))
            while cfill:
                cfill.pop(0)()


_NC = None


def _get_nc():
    global _NC
    if _NC is None:
        _NC = _build_nc()
    return _NC


def _host_prep(input_sequences, Wq, bq, Wk, bk, Wv, bv, Wo, bo):
    """Build per-core input maps (bf16 conversion + pre-gather on host)."""
    bf = ml_dtypes.bfloat16
    x = np.asarray(input_sequences, dtype=np.float32)
    cm = np.full((128, 896), NEG, dtype=np.float32)
    kk = np.arange(128)[:, None]
    uu = np.arange(896)[None, :]
    cm[kk <= uu - 384] = 0.0

    in_maps = []
    for c in range(8):
        b, g = divmod(c, 2)
        sl = slice(g * DHG, (g + 1) * DHG)
        wq_c = np.ascontiguousarray(
            np.asarray(Wq[:, sl], dtype=np.float32)
            .reshape(KT, 128, HPC, 128).transpose(2, 1, 0, 3).reshape(DHG, D)
        ).astype(bf)
        wk_c = np.ascontiguousarray(
            np.asarray(Wk[:, sl], dtype=np.float32)
            .reshape(KT, 128, HPC, 128).transpose(2, 1, 0, 3).reshape(DHG, D)
        ).astype(bf)
        wv_c = np.ascontiguousarray(Wv[:, sl]).astype(np.float32).astype(bf)
        wo_c = np.ascontiguousarray(
            np.asarray(Wo[sl, :], dtype=np.float32)
            .reshape(HPC, 128, D).transpose(1, 0, 2)
        ).astype(bf)
        in_maps.append({
            "xT": np.ascontiguousarray(x[b].T).astype(bf),
            "wq": wq_c,
            "wk": wk_c,
            "wv": wv_c,
            "wo": wo_c,
            "bqT": np.ascontiguousarray(
                np.asarray(bq[sl], dtype=np.float32).reshape(HPC, 128).T
            ),
            "bkT": np.ascontiguousarray(
                np.asarray(bk[sl], dtype=np.float32).reshape(HPC, 128).T
            ),
            "bvb": np.ascontiguousarray(
                np.broadcast_to(np.asarray(bv[sl], dtype=np.float32), (128, DHG))
            ),
            "cmask": cm,
        })
    return in_maps


def kernel(input_sequences, Wq, bq, Wk, bk, Wv, bv, Wo, bo, _trace=False):
    nc = _get_nc()
    in_maps = _host_prep(input_sequences, Wq, bq, Wk, bk, Wv, bv, Wo, bo)
    res = run_bass_kernel_spmd(nc, in_maps, list(range(8)), trace=_trace)
    bo32 = np.asarray(bo, dtype=np.float32)
    out = np.empty((B, S, D), dtype=np.float32)
    for b in range(B):
        out[b] = res.results[2 * b]["out"] + res.results[2 * b + 1]["out"] + bo32
    if _trace:
        kernel.last_exec_time_ns = res.exec_time_ns
    return out


# revision 4
# speedup vs baseline: 1.0127x; 1.0019x over previous
"""Causal multi-head attention on 8 trn2 NeuronCores — v2 (all-bf16, no spills).

Problem: B=4, S=2048, D=2048, H=16 heads, head_dim=128, causal softmax,
torch-style Linear projections (W stored [in, out]).

Sharding: core c handles batch b = c//2 and head-group g = c%2
(8 heads = 1024 output columns of Wq/Wk/Wv, 1024 rows of Wo).
Each core produces a partial output [S, D]; host sums the two
head-group partials per batch and adds bo.

v2 design (vs v1): all operands bf16 (error budget allows: sim 3.6e-3
vs 2e-2 gate), which halves SBUF footprint so Q^T/K^T/V stay fully
SBUF-resident — no DRAM spill/reload.  The output projection is fused
into the attention loop per 512-wide q-chunk (ctx tiles consumed from
SBUF), with its matmuls interleaved between heads of the next q-chunk
as PE filler during ACT-exp waits.  Exp is batched over tile pairs
[128, 2, 512] to halve ACT instruction count.
"""

import numpy as np
import ml_dtypes

import concourse.bass as bass
import concourse.mybir as mybir
import concourse.tile as tile
from concourse import bacc
from concourse.bass_utils import run_bass_kernel_spmd

B = 4
S = 2048
D = 2048
H = 16
DH = 128
HPC = 8          # heads per core
DHG = HPC * DH   # 1024: head-group width per core
KT = D // 128    # 16 k-tiles over the model dim
ST = S // 128    # 16 s-tiles
QC = S // 512    # 4 q-chunks
SCALE = 1.0 / np.sqrt(DH)
NEG = -1.0e30

F32 = mybir.dt.float32
BF16 = mybir.dt.bfloat16
AF = mybir.ActivationFunctionType


def _build_nc():
    nc = bacc.Bacc(None, target_bir_lowering=False)

    xT = nc.declare_dram_parameter("xT", [D, S], BF16, isOutput=False)
    # wq/wk host-pregathered to [HPC*128, KT*128]: row t*128+p, col n*128+m
    # = Wq[n*128+p, t*128+m] so each head-tile's weights DMA contiguously
    wq = nc.declare_dram_parameter("wq", [DHG, D], BF16, isOutput=False)
    wk = nc.declare_dram_parameter("wk", [DHG, D], BF16, isOutput=False)
    wv = nc.declare_dram_parameter("wv", [D, DHG], BF16, isOutput=False)
    wo = nc.declare_dram_parameter("wo", [128, HPC, D], BF16, isOutput=False)
    bqT = nc.declare_dram_parameter("bqT", [128, HPC], F32, isOutput=False)
    bkT = nc.declare_dram_parameter("bkT", [128, HPC], F32, isOutput=False)
    bvb = nc.declare_dram_parameter("bvb", [128, DHG], F32, isOutput=False)
    cmask = nc.declare_dram_parameter("cmask", [128, 896], F32, isOutput=False)
    out = nc.declare_dram_parameter("out", [S, D], F32, isOutput=True)

    with tile.TileContext(nc) as tc:
        _emit(nc, tc, xT, wq, wk, wv, wo, bqT, bkT, bvb, cmask, out)
    nc.compile()
    return nc


def _emit(nc, tc, xT, wq, wk, wv, wo, bqT, bkT, bvb, cmask, out):
    with (
        tc.tile_pool(name="const", bufs=1) as const,
        tc.tile_pool(name="persist", bufs=1) as persist,
    ):
        # small consts needed early
        bq_sb = const.tile([128, HPC], F32)
        nc.sync.dma_start(out=bq_sb, in_=bqT[:, :])
        bk_sb = const.tile([128, HPC], F32)
        nc.sync.dma_start(out=bk_sb, in_=bkT[:, :])
        ones_bf = const.tile([128, 128], BF16)
        nc.vector.memset(ones_bf, 1.0)

        qt_all = persist.tile([128, HPC, S], BF16)
        kt_all = persist.tile([128, HPC, S], BF16)
        v_all = persist.tile([128, ST, DHG], BF16)
        wo_sb = persist.tile([128, HPC, D], BF16)
        cm_sb = const.tile([128, 896], F32)
        bv_sb = const.tile([128, DHG], F32)

        # ---------------- Phase A: projections, SBUF-resident -------------
        wv_r = wv.rearrange("(n p) m -> p n m", p=128)

        with (
            tc.tile_pool(name="xts", bufs=3) as xtp,
            tc.tile_pool(name="wqk", bufs=2) as wqk,
            tc.tile_pool(name="wvp", bufs=2) as wvp,
            tc.tile_pool(name="apsum", bufs=8, space="PSUM") as aps,
        ):
            for sh in range(2):
                s0 = sh * (S // 2)
                seq = [(w, b, dst, t)
                       for w, b, dst in ((wq, bq_sb, qt_all), (wk, bk_sb, kt_all))
                       for t in range(HPC)]
                w_tiles = {}

                def w_prefetch(i):
                    if i < len(seq):
                        w, _, _, t = seq[i]
                        w_sb = wqk.tile([128, KT, 128], BF16, tag="wqk",
                                        name=f"w_sb{i % 2}")
                        nc.sync.dma_start(
                            out=w_sb,
                            in_=w[t * 128 : (t + 1) * 128, :]
                            .rearrange("p (n m) -> p n m", m=128),
                        )
                        w_tiles[i] = w_sb

                w_prefetch(0)

                xt_lo = xtp.tile([128, 8, S // 2], BF16, tag="xts")
                xt_hi = xtp.tile([128, 8, S // 2], BF16, tag="xts")

                def xt_blk(kd):
                    t = xt_lo if kd < 8 else xt_hi
                    return t[:, kd % 8, :]

                for kd in range(KT):
                    nc.sync.dma_start(
                        out=xt_blk(kd),
                        in_=xT[kd * 128 : (kd + 1) * 128, s0 : s0 + S // 2],
                    )
                if sh == 0:
                    # bv needed by the V staging later this half
                    nc.sync.dma_start(out=bv_sb, in_=bvb[:, :])

                # Q^T and K^T: psum[dh 128, s 512] = sum_kd Wblk^T @ xTblk,
                # bias-add on ACT straight into the resident buffers
                for i, (w, b_sb, dst, t) in enumerate(seq):
                    w_sb = w_tiles.pop(i)
                    w_prefetch(i + 1)
                    for sc in range(2):
                        psum = aps.tile([128, 512], F32, tag="apsum", name="qk_ps")
                        for kd in range(KT):
                            nc.tensor.matmul(
                                psum,
                                w_sb[:, kd, :],
                                xt_blk(kd)[:, sc * 512 : (sc + 1) * 512],
                                start=(kd == 0),
                                stop=(kd == KT - 1),
                            )
                        nc.scalar.activation(
                            out=dst[:, t, s0 + sc * 512 : s0 + (sc + 1) * 512],
                            in_=psum,
                            func=AF.Identity,
                            bias=b_sb[:, t : t + 1],
                        )

                # V: psum[s 128, dh 512] = sum_kd xTblk^T @ Wvblk
                for t2 in range(2):
                    wv_lo = wvp.tile([128, 8, 512], BF16, tag="wvp", name="wv_lo")
                    nc.sync.dma_start(
                        out=wv_lo, in_=wv_r[:, 0:8, t2 * 512 : (t2 + 1) * 512]
                    )
                    wv_hi = wvp.tile([128, 8, 512], BF16, tag="wvp", name="wv_hi")
                    nc.sync.dma_start(
                        out=wv_hi, in_=wv_r[:, 8:16, t2 * 512 : (t2 + 1) * 512]
                    )
                    psums = [
                        aps.tile([128, 512], F32, tag="apsum", name=f"vps{si}")
                        for si in range(8)
                    ]
                    for kd in range(KT):
                        wv_blk = wv_lo if kd < 8 else wv_hi
                        for si in range(8):
                            nc.tensor.matmul(
                                psums[si],
                                xt_blk(kd)[:, si * 128 : (si + 1) * 128],
                                wv_blk[:, kd % 8, :],
                                start=(kd == 0),
                                stop=(kd == KT - 1),
                            )
                    for si in range(8):
                        nc.vector.tensor_tensor(
                            out=v_all[:, sh * 8 + si, t2 * 512 : (t2 + 1) * 512],
                            in0=psums[si],
                            in1=bv_sb[:, t2 * 512 : (t2 + 1) * 512],
                            op=mybir.AluOpType.add,
                        )

            # mask needed only at phase B start; queue behind the bulk loads
            nc.sync.dma_start(out=cm_sb, in_=cmask[:, :])

        # ---------------- Phase B + fused C, per q-chunk ---------------------
        with (
            tc.tile_pool(name="msk", bufs=3) as mpool,
            tc.tile_pool(name="ptile", bufs=4) as ppool,
            tc.tile_pool(name="psum8", bufs=3) as spool,
            tc.tile_pool(name="ctx", bufs=2) as ctxp,
            tc.tile_pool(name="rcp", bufs=2) as rcpool,
            tc.tile_pool(name="ostage", bufs=4) as ost,
            tc.tile_pool(name="pscore", bufs=3, space="PSUM") as pscore,
            tc.tile_pool(name="pcd", bufs=1, space="PSUM") as pcd,
        ):
            for hh in range(HPC):
                nc.sync.dma_start(out=wo_sb[:, hh, :], in_=wo[:, hh, :])

            cfill = []  # deferred phase-C psum jobs of the previous q-chunk
            cslot = []  # half-used pscore pair tile shared by job pairs

            def c_job(ctx_t, st, st_rel, ncol):
                def run():
                    # C psums ride the pscore pool, two jobs per pair tile
                    if cslot:
                        psC = cslot.pop()
                    else:
                        pt = pscore.tile([128, 2, 512], F32, tag="pscore")
                        cslot.append(pt[:, 1, :])
                        psC = pt[:, 0, :]
                    for hh in range(HPC):
                        nc.tensor.matmul(
                            psC,
                            ctx_t[:, hh, st_rel * 128 : (st_rel + 1) * 128],
                            wo_sb[:, hh, ncol * 512 : (ncol + 1) * 512],
                            start=(hh == 0),
                            stop=(hh == HPC - 1),
                        )
                    o_sb = ost.tile([128, 512], F32, tag="ostage")
                    nc.scalar.activation(out=o_sb, in_=psC, func=AF.Copy)
                    # alternate store queues so the final drain isn't gated
                    # on a single DMA engine
                    eng = nc.gpsimd if ncol % 2 == 0 else nc.sync
                    eng.dma_start(
                        out=out[
                            st * 128 : (st + 1) * 128,
                            ncol * 512 : (ncol + 1) * 512,
                        ],
                        in_=o_sb,
                    )
                return run

            def emit_head(qc, h, ctx_all):
                # for large chunks, pre-reduce P pairs on DVE so one
                # ones-matmul per pair suffices for the denominator
                pair_den = qc >= 1
                pairs = [
                    (4 * qc, 0, 4 * qc + 1, 128, True),
                    (4 * qc + 2, 256, 4 * qc + 3, 384, True),
                ] + [(2 * i, 0, 2 * i + 1, 0, False) for i in range(2 * qc)]
                qt_c = qt_all[:, h, qc * 512 : (qc + 1) * 512]
                psum_cd = pcd.tile([128, 2, 512], F32, tag="pcd")

                def scores_exp(pr):
                    ka, offa, kb, offb, diag = pr
                    ps = pscore.tile([128, 2, 512], F32, tag="pscore")
                    nc.tensor.matmul(
                        ps[:, 0, offa:],
                        kt_all[:, h, ka * 128 : (ka + 1) * 128],
                        qt_c[:, offa:],
                        start=True, stop=True,
                    )
                    nc.tensor.matmul(
                        ps[:, 1, offb:],
                        kt_all[:, h, kb * 128 : (kb + 1) * 128],
                        qt_c[:, offb:],
                        start=True, stop=True,
                    )
                    p_t = ppool.tile([128, 2, 512], BF16, tag="p_t")
                    if diag:
                        ja, jb = ka - 4 * qc, kb - 4 * qc
                        msk = mpool.tile([128, 2, 512], F32, tag="msk")
                        nc.vector.tensor_tensor(
                            out=msk[:, 0, offa:],
                            in0=ps[:, 0, offa:],
                            in1=cm_sb[:, 384 + offa - 128 * ja : 896 - 128 * ja],
                            op=mybir.AluOpType.add,
                        )
                        nc.vector.tensor_tensor(
                            out=msk[:, 1, offb:],
                            in0=ps[:, 1, offb:],
                            in1=cm_sb[:, 384 + offb - 128 * jb : 896 - 128 * jb],
                            op=mybir.AluOpType.add,
                        )
                        nc.vector.memset(msk[:, 1, offa:offb], NEG)
                        src = msk
                    else:
                        src = ps
                    nc.scalar.activation(
                        out=p_t[:, :, offa:],
                        in_=src[:, :, offa:],
                        func=AF.Exp,
                        scale=float(SCALE),
                    )
                    if pair_den:
                        s_t = spool.tile([128, 512], BF16, tag="psum8")
                        nc.vector.tensor_tensor(
                            out=s_t[:, offa:],
                            in0=p_t[:, 0, offa:],
                            in1=p_t[:, 1, offa:],
                            op=mybir.AluOpType.add,
                        )
                    else:
                        s_t = None
                    return p_t, s_t

                def ctx_mm(pr, p_t, s_t, first, last):
                    ka, offa, kb, offb, _ = pr
                    nc.tensor.matmul(
                        psum_cd[:, 0, offa:],
                        v_all[:, ka, h * 128 : (h + 1) * 128],
                        p_t[:, 0, offa:],
                        start=first, stop=False,
                    )
                    if not pair_den:
                        nc.tensor.matmul(
                            psum_cd[:, 1, offa:],
                            ones_bf,
                            p_t[:, 0, offa:],
                            start=first, stop=False,
                        )
                    nc.tensor.matmul(
                        psum_cd[:, 0, offb:],
                        v_all[:, kb, h * 128 : (h + 1) * 128],
                        p_t[:, 1, offb:],
                        start=False, stop=last,
                    )
                    if pair_den:
                        nc.tensor.matmul(
                            psum_cd[:, 1, offa:],
                            ones_bf,
                            s_t[:, offa:],
                            start=first, stop=last,
                        )
                    else:
                        nc.tensor.matmul(
                            psum_cd[:, 1, offb:],
                            ones_bf,
                            p_t[:, 1, offb:],
                            start=False, stop=last,
                        )

                prev = None
                for i, pr in enumerate(pairs):
                    p_t, s_t = scores_exp(pr)
                    if prev is not None:
                        ctx_mm(prev[0], prev[1], prev[2],
                               first=(i == 1), last=False)
                    prev = (pr, p_t, s_t)
                ctx_mm(prev[0], prev[1], prev[2],
                       first=(len(pairs) == 1), last=True)

                recip = rcpool.tile([128, 512], F32, tag="rcp")
                nc.vector.reciprocal_approx_fast(out=recip, in_=psum_cd[:, 1, :])
                nc.vector.tensor_tensor(
                    out=ctx_all[:, h, :],
                    in0=psum_cd[:, 0, :],
                    in1=recip,
                    op=mybir.AluOpType.mult,
                )

            for qc in (2, 3, 1, 0):
                ctx_all = ctxp.tile([128, HPC, 512], BF16, tag="ctx", name="ctxl")
                for h in range(HPC):
                    emit_head(qc, h, ctx_all)
                    for _ in range(2):
                        if cfill:
                            cfill.pop(0)()
                for st_rel in range(4):
                    st = 4 * qc + st_rel
                    for ncol in range(4):
                        cfill.append(c_job(ctx_all, st, st_rel, ncol))
            while cfill:
                cfill.pop(0)()


_NC = None


def _get_nc():
    global _NC
    if _NC is None:
        _NC = _build_nc()
    return _NC


def _host_prep(input_sequences, Wq, bq, Wk, bk, Wv, bv, Wo, bo):
    """Build per-core input maps (bf16 conversion + pre-gather on host)."""
    bf = ml_dtypes.bfloat16
    x = np.asarray(input_sequences, dtype=np.float32)
    cm = np.full((128, 896), NEG, dtype=np.float32)
    kk = np.arange(128)[:, None]
    uu = np.arange(896)[None, :]
    cm[kk <= uu - 384] = 0.0

    in_maps = []
    for c in range(8):
        b, g = divmod(c, 2)
        sl = slice(g * DHG, (g + 1) * DHG)
        wq_c = np.ascontiguousarray(
            np.asarray(Wq[:, sl], dtype=np.float32)
            .reshape(KT, 128, HPC, 128).transpose(2, 1, 0, 3).reshape(DHG, D)
        ).astype(bf)
        wk_c = np.ascontiguousarray(
            np.asarray(Wk[:, sl], dtype=np.float32)
            .reshape(KT, 128, HPC, 128).transpose(2, 1, 0, 3).reshape(DHG, D)
        ).astype(bf)
        wv_c = np.ascontiguousarray(Wv[:, sl]).astype(np.float32).astype(bf)
        wo_c = np.ascontiguousarray(
            np.asarray(Wo[sl, :], dtype=np.float32)
            .reshape(HPC, 128, D).transpose(1, 0, 2)
        ).astype(bf)
        in_maps.append({
            "xT": np.ascontiguousarray(x[b].T).astype(bf),
            "wq": wq_c,
            "wk": wk_c,
            "wv": wv_c,
            "wo": wo_c,
            "bqT": np.ascontiguousarray(
                np.asarray(bq[sl], dtype=np.float32).reshape(HPC, 128).T
            ),
            "bkT": np.ascontiguousarray(
                np.asarray(bk[sl], dtype=np.float32).reshape(HPC, 128).T
            ),
            "bvb": np.ascontiguousarray(
                np.broadcast_to(np.asarray(bv[sl], dtype=np.float32), (128, DHG))
            ),
            "cmask": cm,
        })
    return in_maps


def kernel(input_sequences, Wq, bq, Wk, bk, Wv, bv, Wo, bo, _trace=False):
    nc = _get_nc()
    in_maps = _host_prep(input_sequences, Wq, bq, Wk, bk, Wv, bv, Wo, bo)
    res = run_bass_kernel_spmd(nc, in_maps, list(range(8)), trace=_trace)
    bo32 = np.asarray(bo, dtype=np.float32)
    out = np.empty((B, S, D), dtype=np.float32)
    for b in range(B):
        out[b] = res.results[2 * b]["out"] + res.results[2 * b + 1]["out"] + bo32
    if _trace:
        kernel.last_exec_time_ns = res.exec_time_ns
    return out
